# revision 1
# baseline (speedup 1.0000x reference)
"""Trainium2 Bass kernel for a 3-layer GCN binary graph classifier.

Self-contained: takes the FULL inputs of reference.setup_inputs(), shards
across 8 NeuronCores internally, returns the FULL [64,1] output.

Algorithm (validated against the jax reference to ~2e-7 rel in numpy):
  - destination-node sharding: core c owns nodes [c*6250, (c+1)*6250)
  - per layer: M' = dinv * (h @ W) computed on the owner core (fp32),
    split into bf16 hi/lo (hi+lo ~= fp32; bf16*bf16 products are exact on
    the PE, accumulated in fp32 PSUM), AllGathered into a shared DRAM
    table [NP, 256]
  - each core dma_gathers its in-edges' rows (512B/edge) and scatter-adds
    via one-hot matmuls (one-hot built on DVE by iota==slot compare),
    feature-major PSUM [128f x 128dst]
  - u = dinv[dst] * psum; BN batch stats via tiny AllReduce; BN+ReLU fused
    on the scalar engine (the conv bias b cancels exactly inside BN)
  - mean-pool via one-hot matmul per graph + AllReduce; sigmoid readout.
"""

import dataclasses
import os

import numpy as np
import ml_dtypes

import concourse.bass as bass
import concourse.bacc as bacc
import concourse.mybir as mybir
from concourse.library_config import mlp as _mlp_lib

BF16 = ml_dtypes.bfloat16
F32 = mybir.dt.float32
BF = mybir.dt.bfloat16
I16 = mybir.dt.int16
AF = mybir.ActivationFunctionType
ALU = mybir.AluOpType

H = 128
EPS = 1e-5
P_CORES = 8


def _ap3(ap, ins_at, pair):
    """Insert [step,count] pair into a 2D AP at position ins_at (1 or 2)."""
    aps = list(ap.ap)
    aps.insert(ins_at, list(pair))
    return dataclasses.replace(ap, ap=aps)


# ---------------------------------------------------------------------------
# host prep
# ---------------------------------------------------------------------------

def _prep(x, edge_index, batch, P, G):
    N, D = x.shape
    assert D == H and N % P == 0
    S = N // P
    T = (S + 127) // 128
    SP = T * 128
    NP = P * SP
    HALF = NP // 2
    assert HALF - 1 < 32768

    src = np.asarray(edge_index[0], dtype=np.int64)
    dst = np.asarray(edge_index[1], dtype=np.int64)
    batch = np.asarray(batch, dtype=np.int64)

    deg = np.bincount(dst, minlength=N).astype(np.float32) + 1.0
    dinv = (1.0 / np.sqrt(deg)).astype(np.float32)

    nodes = np.arange(N, dtype=np.int64)
    owner = nodes // S
    ln = nodes - owner * S
    rowmap = owner * SP + (ln % 128) * T + (ln // 128)

    src_all = np.concatenate([src, nodes])
    dst_all = np.concatenate([dst, nodes])
    srow = rowmap[src_all]
    half = (srow >= HALF).astype(np.int64)

    c_of = dst_all // S
    ld = dst_all - c_of * S
    t_of = ld // 128
    slot_of = ld % 128

    key = ((c_of * T) + t_of) * 2 + half
    order = np.argsort(key, kind="stable")
    key_s = key[order]
    srow_s = srow[order]
    slot_s = slot_of[order]

    ngroups = P * T * 2
    starts = np.searchsorted(key_s, np.arange(ngroups))
    ends = np.searchsorted(key_s, np.arange(ngroups) + 1)
    cnt = (ends - starts).reshape(P, T, 2)

    CA = np.ceil(cnt[:, :, 0].max(axis=0) / 128).astype(np.int64)
    CB = np.ceil(cnt[:, :, 1].max(axis=0) / 128).astype(np.int64)
    C_tot = int((CA + CB).sum())
    idx_cols = C_tot * 8

    tiles = []
    icol = 0
    ccol = 0
    for t in range(T):
        rec = []
        for hh, C in ((0, int(CA[t])), (1, int(CB[t]))):
            rec.append((C, icol, ccol))
            icol += C * 8
            ccol += C
        tiles.append((rec[0][0], rec[1][0], rec[0][1], rec[1][1],
                      rec[0][2], rec[1][2]))

    per_core = []
    for c in range(P):
        idxs = np.zeros((128, idx_cols), dtype=np.int16)
        slots = np.full((128, C_tot), 1000.0, dtype=np.float32)
        for t in range(T):
            CAt, CBt, icolA, icolB, ccolA, ccolB = tiles[t]
            for hh, C, ic, cc in ((0, CAt, icolA, ccolA), (1, CBt, icolB, ccolB)):
                if C == 0:
                    continue
                g = starts[(c * T + t) * 2 + hh]
                e = ends[(c * T + t) * 2 + hh]
                n = e - g
                arr_i = np.zeros(C * 128, dtype=np.int64)
                arr_s = np.full(C * 128, 1000.0, dtype=np.float32)
                arr_i[:n] = srow_s[g:e] - (HALF if hh else 0)
                arr_s[:n] = slot_s[g:e]
                w = arr_i.reshape(C * 8, 16).T.astype(np.int16)
                idxs[:, ic:ic + C * 8] = np.tile(w, (8, 1))
                slots[:, cc:cc + C] = arr_s.reshape(C, 128).T

        xst = np.zeros((SP, D), dtype=np.float32)
        lnn = np.arange(S)
        xst[(lnn % 128) * T + lnn // 128] = np.asarray(x[c * S:(c + 1) * S],
                                                       np.float32)
        dinv_col = np.zeros((128, T), dtype=np.float32)
        batch_col = np.full((128, T), float(G), dtype=np.float32)
        dsh = dinv[c * S:(c + 1) * S]
        bsh = batch[c * S:(c + 1) * S].astype(np.float32)
        dinv_col[lnn % 128, lnn // 128] = dsh
        batch_col[lnn % 128, lnn // 128] = bsh
        dinv_b = np.zeros((SP,), dtype=np.float32)
        dinv_b[:S] = dsh
        dinv_bcast = np.broadcast_to(dinv_b, (128, SP)).copy()

        per_core.append(dict(idxs=idxs, slots=slots, xs=xst, dinv_col=dinv_col,
                             batch_col=batch_col, dinv_bcast=dinv_bcast))

    counts = np.bincount(batch, minlength=G).astype(np.float32)
    cinv = (1.0 / np.maximum(counts, 1.0)).astype(np.float32)

    meta = dict(N=N, P=P, S=S, T=T, SP=SP, NP=NP, HALF=HALF, G=G,
                C_tot=C_tot, idx_cols=idx_cols, tiles=tiles)
    return meta, per_core, cinv


def _pack_cpack(meta, pc, params, cinv):
    """Pack all small f32 inputs into one [128, CK] tensor; returns (arr, off)."""
    T, SP, C_tot, G = meta["T"], meta["SP"], meta["C_tot"], meta["G"]
    cols = {}
    pos = 0

    def add(name, n):
        nonlocal pos
        cols[name] = pos
        pos += n

    add("ident", 128)
    add("iota128", 128)
    add("iotaG", G)
    add("w0", 128)
    add("w1", 128)
    add("w2", 128)
    add("wout", 1)
    add("g0", 1); add("be0", 1); add("g1", 1); add("be1", 1)
    add("g2", 1); add("be2", 1)
    add("cinv", 1)
    add("boutv", 1)
    add("dinv_col", T)
    add("batch_col", T)
    add("dinv_bcast", SP)
    add("slots", C_tot)
    CK = pos

    a = np.zeros((128, CK), dtype=np.float32)
    a[:, cols["ident"]:cols["ident"] + 128] = np.eye(128, dtype=np.float32)
    a[:, cols["iota128"]:cols["iota128"] + 128] = np.broadcast_to(
        np.arange(128, dtype=np.float32), (128, 128))
    a[:, cols["iotaG"]:cols["iotaG"] + G] = np.broadcast_to(
        np.arange(G, dtype=np.float32), (128, G))
    for i in range(3):
        a[:, cols[f"w{i}"]:cols[f"w{i}"] + 128] = np.asarray(params[f"W{i}"],
                                                             np.float32)
        a[:, cols[f"g{i}"]] = np.asarray(params[f"g{i}"], np.float32)
        a[:, cols[f"be{i}"]] = np.asarray(params[f"be{i}"], np.float32)
    a[:, cols["wout"]] = np.asarray(params["Wout"], np.float32)[:, 0]
    a[:meta["G"], cols["cinv"]] = cinv
    a[:meta["G"], cols["boutv"]] = float(np.asarray(params["bout"],
                                                    np.float32)[0])
    a[:, cols["dinv_col"]:cols["dinv_col"] + T] = pc["dinv_col"]
    a[:, cols["batch_col"]:cols["batch_col"] + T] = pc["batch_col"]
    a[:, cols["dinv_bcast"]:cols["dinv_bcast"] + SP] = pc["dinv_bcast"]
    a[:, cols["slots"]:cols["slots"] + C_tot] = pc["slots"]
    return a, cols, CK


# ---------------------------------------------------------------------------
# device program
# ---------------------------------------------------------------------------

def build_nc(meta, cols, CK, reps=1, no_cc=False, no_gather=False):
    N, P, T, SP, NP, HALF, G = (meta[k] for k in
                                ("N", "P", "T", "SP", "NP", "HALF", "G"))
    tiles = meta["tiles"]
    C_tot, idx_cols = meta["C_tot"], meta["idx_cols"]
    Cmax = max(ca + cb for ca, cb, *_ in tiles)
    # SWDGE descriptor ring holds ~1024 descs; cap idxs per dma_gather call
    GCAP = 7  # chunks of 128 idxs per call
    calls_per_tile = [-(-ca // GCAP) + -(-cb // GCAP)
                      for ca, cb, *_ in tiles]

    nc = bacc.Bacc("TRN2", num_devices=P)
    rg = [list(range(P))]

    cpack_d = nc.declare_dram_parameter("cpack", [128, CK], F32, isOutput=False)
    xs_d = nc.declare_dram_parameter("xs", [SP, H], F32, isOutput=False)
    idxs_d = nc.declare_dram_parameter("idxs", [128, idx_cols], I16,
                                       isOutput=False)
    out_d = nc.declare_dram_parameter("out", [G, 1], F32, isOutput=True)

    cc_in = nc.dram_tensor("cc_in", [SP, 128], F32)
    mfull = nc.dram_tensor("mfull", [NP, 128], F32, addr_space="Shared")
    ar_in = nc.dram_tensor("ar_in", [128, 2], F32)
    ar_out = nc.dram_tensor("ar_out", [128, 2], F32, addr_space="Shared")
    ar2_in = nc.dram_tensor("ar2_in", [128, G], F32)
    ar2_out = nc.dram_tensor("ar2_out", [128, G], F32, addr_space="Shared")

    import contextlib
    es = contextlib.ExitStack()

    def sb(name, shape, dt):
        return es.enter_context(nc.sbuf_tensor(name, shape, dt))

    def ps(name, shape):
        return es.enter_context(nc.psum_tensor(name, shape, F32))

    def sem(name):
        return es.enter_context(nc.semaphore(name))

    with es:
        cp = sb("cp", [128, CK], F32)
        xsb = sb("xsb", [128, T * H], F32)
        hT = sb("hT", [128, SP], F32)
        uT = sb("uT", [128, SP], F32)
        mps = sb("mps", [128, T * 128], F32)
        idxs_sb = sb("idxs_sb", [128, idx_cols], I16)
        gath = sb("gath", [128, 3 * Cmax * 128], F32)
        vhot = sb("vhot", [128, 2 * Cmax * 128], F32)
        scr2 = sb("scr2", [128, 2 * 128], F32)
        st1 = sb("st1", [128, T], F32)
        NG8 = -(-T // 8)
        st2 = sb("st2", [128, NG8], F32)
        stp = sb("stp", [128, 2], F32)
        sta = sb("sta", [128, 2], F32)
        bnp = sb("bnp", [128, 8], F32)
        h3n = sb("h3n", [128, 2 * 128], F32)
        bsel = sb("bsel", [128, 2 * G], F32)
        poos = sb("poos", [128, G], F32)
        pooa = sb("pooa", [128, G], F32)
        outs = sb("outs", [64, 1], F32)

        psA = ps("psA", [128, 2, 512])
        psB = ps("psB", [128, 2, 512])
        psP = ps("psP", [128, 512])
        psO = ps("psO", [128, 512])

        s_in = sem("s_in")
        s_tp = sem("s_tp")
        s_xc = sem("s_xc")
        s_pet = sem("s_pet")
        s_hi = sem("s_hi")
        s_mp = sem("s_mp")
        s_st = sem("s_st")
        s_cc = sem("s_cc")
        s_g = [sem("s_g0"), sem("s_g1"), sem("s_g2")]
        s_vh = sem("s_vh")
        s_pe = sem("s_pe")
        s_uc = sem("s_uc")
        s_uc2 = sem("s_uc2")
        s_sr = sem("s_sr")
        s_ld = sem("s_ld")
        s_bp1 = sem("s_bp1")
        s_bp2 = sem("s_bp2")
        s_bp3 = sem("s_bp3")
        s_bn = sem("s_bn")
        s_tp2 = sem("s_tp2")
        s_h3 = sem("s_h3")
        s_pool = sem("s_pool")
        s_pc = sem("s_pc")
        s_ro = sem("s_ro")
        s_sig = sem("s_sig")

        def col(name, n=1):
            return cp[:, cols[name]:cols[name] + n]

        ident_ap = col("ident", 128)
        iota128_ap = col("iota128", 128)
        iotaG_ap = col("iotaG", G)
        w_ap = [col("w0", 128), col("w1", 128), col("w2", 128)]
        wout_ap = cp[:, cols["wout"]:cols["wout"] + 1]
        cinv_ap = cp[0:G, cols["cinv"]:cols["cinv"] + 1]
        boutv_ap = cp[0:G, cols["boutv"]:cols["boutv"] + 1]

        gath3 = gath[:, :].rearrange("p (s c e) -> p s c e", s=3, e=128)
        vhot3 = vhot[:, :].rearrange("p (s c e) -> p s c e", s=2, e=128)
        mps3 = mps[:, :].rearrange("p (t e) -> p t e", e=128)

        # per-slot gather sem values: slot of global tile gidx = gidx % 3;
        # g_slot_val[gidx] = 16 * (total calls issued on that slot's sem
        # through gidx) — safe wait target under unordered DMA completion.
        RL = 3 * reps  # total layer-slots
        g_slot_val = []
        slot_tally = [0, 0, 0]
        for gidx in range(RL * T):
            slot_tally[gidx % 3] += calls_per_tile[gidx % T]
            g_slot_val.append(16 * slot_tally[gidx % 3])

        def gcalls(C):
            """Split C chunks into dma_gather call spans of <= GCAP chunks."""
            return [(c0, min(c0 + GCAP, C)) for c0 in range(0, C, GCAP)]

        CCI = 16 if no_cc else 1  # s_cc increment per collective

        with nc.Block() as block:

            @block.sync
            def _(sync):
                sync.dma_start(out=cp[:, :], in_=cpack_d[:, :]).then_inc(s_in, 16)
                sync.dma_start(
                    out=xsb[:, :],
                    in_=xs_d.ap().rearrange("(p t) f -> p (t f)", p=128),
                ).then_inc(s_in, 16)
                sync.dma_start(out=idxs_sb[:, :], in_=idxs_d[:, :]).then_inc(
                    s_in, 16)
                for rep in range(reps):
                    for l in range(3):
                        LG = rep * 3 + l
                        # M' store -> cc_in
                        sync.wait_ge(s_mp, (LG + 1) * T)
                        sync.dma_start(
                            out=cc_in.ap().rearrange("(p t) f -> p (t f)",
                                                     p=128),
                            in_=mps[:, :],
                        ).then_inc(s_st, 16)
                        # stats store
                        sync.wait_ge(s_sr, LG + 1)
                        sync.dma_start(out=ar_in[:, :],
                                       in_=stp[:, :]).then_inc(s_st, 16)
                        # stats load back after AR
                        sync.wait_ge(s_cc, (rep * 7 + 2 * l + 2) * CCI)
                        sync.dma_start(out=sta[:, :],
                                       in_=ar_out[:, :]).then_inc(s_ld, 16)
                    # pooled store
                    sync.wait_ge(s_pc, rep + 1)
                    sync.dma_start(out=ar2_in[:, :], in_=poos[:, :]).then_inc(
                        s_st, 16)
                    # pooled load after AR
                    sync.wait_ge(s_cc, (rep * 7 + 7) * CCI)
                    sync.dma_start(out=pooa[:, :],
                                   in_=ar2_out[:, :]).then_inc(s_ld, 16)
                    # final out
                    sync.wait_ge(s_sig, rep + 1)
                    sync.dma_start(out=out_d[:, :], in_=outs[:, :]).then_inc(
                        s_st, 16)

            def _cc(gpsimd, kind, op, ins, outs):
                """Collective, or (no_cc timing mode) a local stand-in DMA."""
                if no_cc:
                    return gpsimd.dma_start(
                        out=outs[0].tensor[0:ins[0].shape[0], :],
                        in_=ins[0])
                return gpsimd.collective_compute(
                    kind, op, replica_groups=rg, ins=ins, outs=outs)

            @block.gpsimd
            def _(gpsimd):
                gpsimd.load_library(_mlp_lib)
                gpsimd.wait_ge(s_in, 48)
                for rep in range(reps):
                    for l in range(3):
                        LG = rep * 3 + l
                        # AllGather M'
                        gpsimd.wait_ge(s_st, rep * 128 + 32 * l + 16)
                        _cc(gpsimd, "AllGather", ALU.bypass,
                            [cc_in[:, :]], [mfull[:, :]]).then_inc(s_cc, CCI)
                        gpsimd.wait_ge(s_cc, (rep * 7 + 2 * l + 1) * CCI)
                        for t in range(T):
                            gidx = LG * T + t
                            # one wait per 3-tile group: s_pe >= gidx at
                            # gidx%3==0 covers slots gidx, gidx+1, gidx+2
                            # (each needs only s_pe >= gidx-2..gidx)
                            if gidx >= 3 and gidx % 3 == 0:
                                gpsimd.wait_ge(s_pe, gidx)
                            CAt, CBt, icolA, icolB, _, _ = tiles[t]
                            gslot = gidx % 3
                            for (C, ic, cb, base) in (
                                (CAt, icolA, 0, 0),
                                (CBt, icolB, CAt, HALF),
                            ):
                                for (c0, c1) in gcalls(C):
                                    if no_gather:
                                        continue
                                    gpsimd.dma_gather(
                                        gath3[:, gslot, cb + c0:cb + c1, :],
                                        mfull[base:base + HALF, :],
                                        idxs_sb[:, ic + c0 * 8:ic + c1 * 8],
                                        (c1 - c0) * 128,
                                        (c1 - c0) * 128,
                                        128,
                                    ).then_inc(s_g[gslot], 16)
                        # AllReduce stats
                        gpsimd.wait_ge(s_st, rep * 128 + 32 * l + 32)
                        _cc(gpsimd, "AllReduce", ALU.add,
                            [ar_in[:, :]], [ar_out[:, :]]).then_inc(s_cc, CCI)
                    # pooled AllReduce
                    gpsimd.wait_ge(s_st, rep * 128 + 112)
                    _cc(gpsimd, "AllReduce", ALU.add,
                        [ar2_in[:, :]], [ar2_out[:, :]]).then_inc(s_cc, CCI)

            @block.tensor
            def _(tensor):
                tensor.wait_ge(s_in, 48)
                for rep in range(reps):
                    PB = rep * 5 * T  # psB global sequence base
                    # x-prep: transpose x tiles into psB, DVE copies to hT
                    for t in range(T):
                        if t < 2:
                            if rep > 0:
                                tensor.wait_ge(s_h3, rep * T)
                        else:
                            tensor.wait_ge(s_xc, rep * T + t - 1)
                        tensor.matmul(
                            psB[:, (PB + t) % 2, 0:128],
                            xsb[:, t * H:(t + 1) * H],
                            ident_ap,
                            is_transpose=True,
                        ).then_inc(s_tp, 1)
                    for l in range(3):
                        LG = rep * 3 + l
                        # transform: t_tile = hT_tile.T @ W (node-major psB)
                        for t in range(T):
                            midx = LG * T + t
                            pb = PB + T + l * T + t
                            if l == 0 and t == 0:
                                tensor.wait_ge(s_xc, (rep + 1) * T)
                            if l > 0 and t == 0:
                                tensor.wait_ge(s_bn, LG)
                            if midx >= 2 and midx % 2 == 0:
                                # covers this tile (needs midx-1) AND the
                                # next odd tile (needs midx)
                                tensor.wait_ge(s_mp, midx)
                            tensor.matmul(
                                psB[:, pb % 2, 0:128],
                                hT[:, t * H:(t + 1) * H],
                                w_ap[l],
                            ).then_inc(s_pet, 1)
                        # scatter
                        for t in range(T):
                            gidx = LG * T + t
                            CAt, CBt, *_ = tiles[t]
                            Ct = CAt + CBt
                            if not no_gather:
                                tensor.wait_ge(s_g[gidx % 3],
                                               g_slot_val[gidx])
                            # s_vh >= gidx+1 transitively implies s_uc >= gidx-1
                            # (vhot(gidx) is emitted after ucopy(gidx-2) on DVE)
                            tensor.wait_ge(s_vh, gidx + 1)
                            gslot = gidx % 3
                            vslot = gidx % 2
                            for c in range(Ct):
                                mm = tensor.matmul(
                                    psA[:, gidx % 2, 0:128],
                                    gath3[:, gslot, c, 0:128],
                                    vhot3[:, vslot, c, :],
                                    start=(c == 0), stop=(c == Ct - 1),
                                )
                            mm.then_inc(s_pe, 1)
                    # pooling
                    for t in range(T):
                        pb = PB + 4 * T + t
                        if t == 0:
                            tensor.wait_ge(s_bn, rep * 3 + 3)
                        if t >= 2:
                            tensor.wait_ge(s_h3, rep * T + t - 1)
                        tensor.matmul(
                            psB[:, pb % 2, 0:128],
                            hT[:, t * H:(t + 1) * H],
                            ident_ap,
                            is_transpose=True,
                        ).then_inc(s_tp2, 1)
                        if t >= 1:
                            tensor.wait_ge(s_h3, rep * T + t)
                            tensor.matmul(
                                psP[:, 0:G],
                                h3n[:, 128 * ((t - 1) % 2):
                                    128 * ((t - 1) % 2) + 128],
                                bsel[:, G * ((t - 1) % 2):
                                     G * ((t - 1) % 2) + G],
                                start=(t == 1), stop=False,
                            )
                    tensor.wait_ge(s_h3, (rep + 1) * T)
                    tensor.matmul(
                        psP[:, 0:G],
                        h3n[:, 128 * ((T - 1) % 2):128 * ((T - 1) % 2) + 128],
                        bsel[:, G * ((T - 1) % 2):G * ((T - 1) % 2) + G],
                        start=(T == 1), stop=True,
                    ).then_inc(s_pool, 1)
                    # readout matmul
                    tensor.wait_ge(s_ld, rep * 64 + 64)
                    tensor.matmul(psO[0:G, 0:1], pooa[:, 0:G],
                                  wout_ap).then_inc(s_ro, 1)

            @block.scalar
            def _(scalar):
                for rep in range(reps):
                    PB = rep * 5 * T
                    for l in range(3):
                        LG = rep * 3 + l
                        for t in range(T):
                            midx = LG * T + t
                            pb = PB + T + l * T + t
                            scalar.wait_ge(s_pet, midx + 1)
                            scalar.activation(
                                mps3[:, t, 0:128],
                                psB[:, pb % 2, 0:128],
                                AF.Copy,
                                scale=col("dinv_col", T)[:, t:t + 1],
                            ).then_inc(s_mp, 1)
                        # scatter phase: sumsq per GROUP of 8 tiles (u by DVE);
                        # scratch goes to mps, which is dead during scatter
                        # (M' already stored to DRAM before the AllGather).
                        for gi, t0 in enumerate(range(0, T, 8)):
                            t1 = min(t0 + 8, T)
                            scalar.wait_ge(s_uc, LG * T + t1)
                            scalar.activation(
                                mps3[:, t0:t1, 0:128],
                                uT[:, t0 * H:t1 * H].rearrange(
                                    "p (t f) -> p t f", f=H),
                                AF.Square,
                                accum_out=st2[:, gi:gi + 1],
                            ).then_inc(s_uc2, 1)
                        # BN tail: sqrt, A = sd*g, mA = mu*A
                        scalar.wait_ge(s_bp1, LG + 1)
                        scalar.activation(bnp[:, 5:6], bnp[:, 4:5], AF.Sqrt)
                        scalar.drain()
                        scalar.activation(bnp[:, 6:7], bnp[:, 5:6], AF.Copy,
                                          scale=col(f"g{l}"))
                        scalar.drain()
                        scalar.activation(bnp[:, 7:8], bnp[:, 6:7], AF.Copy,
                                          scale=bnp[:, 0:1]).then_inc(s_bp2, 1)
                        # BN+relu big op
                        scalar.wait_ge(s_bp3, LG + 1)
                        scalar.activation(
                            hT[:, :], uT[:, :], AF.Relu,
                            bias=bnp[:, 3:4], scale=bnp[:, 6:7],
                        ).then_inc(s_bn, 1)
                    # sigmoid readout
                    scalar.wait_ge(s_ro, rep + 1)
                    scalar.activation(outs[:, :], psO[0:G, 0:1], AF.Sigmoid,
                                      bias=boutv_ap, scale=cinv_ap).then_inc(
                                          s_sig, 1)

            @block.vector
            def _(vector):
                vector.wait_ge(s_in, 48)
                for rep in range(reps):
                    PB = rep * 5 * T
                    # x-prep copies psB -> hT
                    for t in range(T):
                        vector.wait_ge(s_tp, rep * T + t + 1)
                        vector.tensor_copy(
                            hT[:, t * H:(t + 1) * H],
                            psB[:, (PB + t) % 2, 0:128]).then_inc(s_xc, 1)
                    for l in range(3):
                        LG = rep * 3 + l
                        # scatter phase: vhot prologue
                        base_pe = LG * T
                        for pv in range(min(2, T)):
                            gidx = base_pe + pv
                            if gidx >= 2:
                                vector.wait_ge(s_pe, gidx - 1)
                            CAt, CBt, _, _, ccolA, _ = tiles[pv]
                            Ct = CAt + CBt
                            vector.tensor_tensor(
                                vhot3[:, gidx % 2, 0:Ct, :],
                                _ap3(iota128_ap, 1, (0, Ct)),
                                _ap3(col("slots", C_tot)[:, ccolA:ccolA + Ct],
                                     2, (0, 128)),
                                op=ALU.is_equal,
                            ).then_inc(s_vh, 1)
                        for t in range(T):
                            gidx = base_pe + t
                            vector.wait_ge(s_pe, gidx + 1)
                            # u = psum * dinv, accumulate sum
                            vector.scalar_tensor_tensor(
                                uT[:, t * H:(t + 1) * H],
                                psA[:, gidx % 2, 0:128],
                                1.0,
                                col("dinv_bcast", SP)[:, t * H:(t + 1) * H],
                                op0=ALU.mult, op1=ALU.mult,
                                accum_out=st1[:, t:t + 1],
                            ).then_inc(s_uc, 1)
                            if t + 2 < T:
                                nt = t + 2
                                CAt, CBt, _, _, ccolA, _ = tiles[nt]
                                Ct = CAt + CBt
                                vector.tensor_tensor(
                                    vhot3[:, (base_pe + nt) % 2, 0:Ct, :],
                                    _ap3(iota128_ap, 1, (0, Ct)),
                                    _ap3(col("slots", C_tot)[:,
                                         ccolA:ccolA + Ct], 2, (0, 128)),
                                    op=ALU.is_equal,
                                ).then_inc(s_vh, 1)
                        # stats reduce
                        vector.drain()
                        vector.wait_ge(s_uc2, (LG + 1) * NG8)
                        vector.reduce_sum(stp[:, 0:1], st1[:, :],
                                          axis=mybir.AxisListType.X)
                        vector.reduce_sum(stp[:, 1:2], st2[:, 0:NG8],
                                          axis=mybir.AxisListType.X).then_inc(
                                              s_sr, 1)
                        # BN params from AR result
                        vector.wait_ge(s_ld, rep * 64 + 16 * (l + 1))
                        vector.tensor_scalar_mul(bnp[:, 0:1], sta[:, 0:1],
                                                 1.0 / N)
                        vector.tensor_scalar_mul(bnp[:, 1:2], sta[:, 1:2],
                                                 1.0 / N)
                        vector.drain()
                        vector.tensor_mul(bnp[:, 2:3], bnp[:, 0:1],
                                          bnp[:, 0:1])
                        vector.drain()
                        vector.tensor_sub(bnp[:, 2:3], bnp[:, 1:2],
                                          bnp[:, 2:3])
                        vector.drain()
                        vector.tensor_scalar_add(bnp[:, 2:3], bnp[:, 2:3],
                                                 EPS)
                        vector.drain()
                        vector.reciprocal(bnp[:, 4:5], bnp[:, 2:3]).then_inc(
                            s_bp1, 1)
                        # B = be - mu*A (scalar computed mA in bnp[:,7:8])
                        vector.wait_ge(s_bp2, LG + 1)
                        vector.tensor_sub(bnp[:, 3:4], col(f"be{l}"),
                                          bnp[:, 7:8]).then_inc(s_bp3, 1)
                    # pooling: copy transposes + bsel
                    for t in range(T):
                        vector.wait_ge(s_tp2, rep * T + t + 1)
                        pb = PB + 4 * T + t
                        vector.tensor_copy(
                            h3n[:, 128 * (t % 2):128 * (t % 2) + 128],
                            psB[:, pb % 2, 0:128])
                        vector.tensor_scalar(
                            bsel[:, G * (t % 2):G * (t % 2) + G],
                            iotaG_ap,
                            col("batch_col", T)[:, t:t + 1],
                            None,
                            op0=ALU.is_equal,
                        ).then_inc(s_h3, 1)
                    vector.wait_ge(s_pool, rep + 1)
                    vector.tensor_copy(poos[:, :],
                                       psP[:, 0:G]).then_inc(s_pc, 1)

        nc.compile()
    return nc


# ---------------------------------------------------------------------------
# entry point
# ---------------------------------------------------------------------------

def kernel(**inputs):
    x = np.asarray(inputs["x"], np.float32)
    edge_index = np.asarray(inputs["edge_index"])
    batch = np.asarray(inputs["batch"])
    G = 64
    P = P_CORES

    meta, per_core, cinv = _prep(x, edge_index, batch, P, G)
    in_maps = []
    cols = CK = None
    for c in range(P):
        cpack, cols, CK = _pack_cpack(meta, per_core[c], inputs, cinv)
        in_maps.append({
            "cpack": cpack,
            "xs": per_core[c]["xs"],
            "idxs": per_core[c]["idxs"],
        })

    nc = build_nc(meta, cols, CK)

    if os.environ.get("GCN_SIM"):
        from concourse import bass_interp
        sim = bass_interp.MultiCoreSim(nc, P)
        for c in range(P):
            for k, v in in_maps[c].items():
                sim.cores[c].tensor(k)[:] = v
        sim.simulate()
        return np.asarray(sim.cores[0].mem_tensor("out"), np.float32)

    from concourse.bass_utils import run_bass_kernel_spmd
    trace = bool(os.environ.get("GCN_TRACE"))
    res = run_bass_kernel_spmd(nc, in_maps, core_ids=list(range(P)),
                               trace=trace)
    if trace and res.exec_time_ns is not None:
        print(f"HW exec time: {res.exec_time_ns} ns")
    return np.asarray(res.results[0]["out"], np.float32)



# revision 27
# speedup vs baseline: 591.5797x; 591.5797x over previous
"""Trainium2 Bass kernel for a 3-layer GCN binary graph classifier.

Self-contained: takes the FULL inputs of reference.setup_inputs(), shards
across 8 NeuronCores internally, returns the FULL [64,1] output.

Algorithm (validated against the jax reference; bf16 message table keeps
the final rel err ~1e-4, far inside the 2e-2 gate):
  - destination-node sharding: core c owns nodes [c*6250, (c+1)*6250)
  - per layer: M' = dinv * (h @ W) computed on the owner core (fp32 PSUM),
    rounded to bf16 and AllGathered into a shared DRAM table [NP, 128]bf16
  - each core dma_gathers its in-edges' rows (256B/edge) across 3 SWDGE
    queues (slot gidx%3 -> queue gidx%3; a DMA-completion semaphore is
    locked to one queue) and scatter-adds via one-hot matmuls (one-hot
    built on DVE by iota==slot compare, bf16), feature-major PSUM
    [128f x 128dst]
  - u = dinv[dst] * psum; BN batch stats via tiny AllReduce; BN+ReLU fused
    on the scalar engine (the conv bias b cancels exactly inside BN)
  - mean-pool via one-hot matmul per graph + AllReduce; sigmoid readout.
"""

import dataclasses
import os

import numpy as np
import ml_dtypes

import concourse.bass as bass
import concourse.bacc as bacc
import concourse.mybir as mybir
from concourse.library_config import mlp as _mlp_lib

BF16 = ml_dtypes.bfloat16
F32 = mybir.dt.float32
BF = mybir.dt.bfloat16
I16 = mybir.dt.int16
AF = mybir.ActivationFunctionType
ALU = mybir.AluOpType

H = 128
EPS = 1e-5
P_CORES = 8


def _ap3(ap, ins_at, pair):
    """Insert [step,count] pair into a 2D AP at position ins_at (1 or 2)."""
    aps = list(ap.ap)
    aps.insert(ins_at, list(pair))
    return dataclasses.replace(ap, ap=aps)


# ---------------------------------------------------------------------------
# host prep
# ---------------------------------------------------------------------------

def _prep(x, edge_index, batch, P, G):
    N, D = x.shape
    assert D == H and N % P == 0
    S = N // P
    T = (S + 127) // 128
    SP = T * 128
    NP = P * SP
    HALF = NP // 2
    assert HALF - 1 < 32768

    src = np.asarray(edge_index[0], dtype=np.int64)
    dst = np.asarray(edge_index[1], dtype=np.int64)
    batch = np.asarray(batch, dtype=np.int64)

    deg = np.bincount(dst, minlength=N).astype(np.float32) + 1.0
    dinv = (1.0 / np.sqrt(deg)).astype(np.float32)

    nodes = np.arange(N, dtype=np.int64)
    owner = nodes // S
    ln = nodes - owner * S
    rowmap = owner * SP + (ln % 128) * T + (ln // 128)

    src_all = np.concatenate([src, nodes])
    dst_all = np.concatenate([dst, nodes])
    srow = rowmap[src_all]
    half = (srow >= HALF).astype(np.int64)

    c_of = dst_all // S
    ld = dst_all - c_of * S
    t_of = ld // 128
    slot_of = ld % 128

    key = ((c_of * T) + t_of) * 2 + half
    order = np.argsort(key, kind="stable")
    key_s = key[order]
    srow_s = srow[order]
    slot_s = slot_of[order]

    ngroups = P * T * 2
    starts = np.searchsorted(key_s, np.arange(ngroups))
    ends = np.searchsorted(key_s, np.arange(ngroups) + 1)
    cnt = (ends - starts).reshape(P, T, 2)

    CA = np.ceil(cnt[:, :, 0].max(axis=0) / 128).astype(np.int64)
    CB = np.ceil(cnt[:, :, 1].max(axis=0) / 128).astype(np.int64)
    C_tot = int((CA + CB).sum())
    idx_cols = C_tot * 8

    tiles = []
    icol = 0
    ccol = 0
    for t in range(T):
        rec = []
        for hh, C in ((0, int(CA[t])), (1, int(CB[t]))):
            rec.append((C, icol, ccol))
            icol += C * 8
            ccol += C
        tiles.append((rec[0][0], rec[1][0], rec[0][1], rec[1][1],
                      rec[0][2], rec[1][2]))

    per_core = []
    for c in range(P):
        idxs = np.zeros((128, idx_cols), dtype=np.int16)
        slots = np.full((128, C_tot), 1000.0, dtype=np.float32)
        for t in range(T):
            CAt, CBt, icolA, icolB, ccolA, ccolB = tiles[t]
            for hh, C, ic, cc in ((0, CAt, icolA, ccolA), (1, CBt, icolB, ccolB)):
                if C == 0:
                    continue
                g = starts[(c * T + t) * 2 + hh]
                e = ends[(c * T + t) * 2 + hh]
                n = e - g
                arr_i = np.zeros(C * 128, dtype=np.int64)
                arr_s = np.full(C * 128, 1000.0, dtype=np.float32)
                arr_i[:n] = srow_s[g:e] - (HALF if hh else 0)
                arr_s[:n] = slot_s[g:e]
                w = arr_i.reshape(C * 8, 16).T.astype(np.int16)
                idxs[:, ic:ic + C * 8] = np.tile(w, (8, 1))
                slots[:, cc:cc + C] = arr_s.reshape(C, 128).T

        xst = np.zeros((SP, D), dtype=np.float32)
        lnn = np.arange(S)
        xst[(lnn % 128) * T + lnn // 128] = np.asarray(x[c * S:(c + 1) * S],
                                                       np.float32)
        dinv_col = np.zeros((128, T), dtype=np.float32)
        batch_col = np.full((128, T), float(G), dtype=np.float32)
        dsh = dinv[c * S:(c + 1) * S]
        bsh = batch[c * S:(c + 1) * S].astype(np.float32)
        dinv_col[lnn % 128, lnn // 128] = dsh
        batch_col[lnn % 128, lnn // 128] = bsh
        dinv_b = np.zeros((SP,), dtype=np.float32)
        dinv_b[:S] = dsh
        dinv_bcast = np.broadcast_to(dinv_b, (128, SP)).copy()

        per_core.append(dict(idxs=idxs, slots=slots, xs=xst, dinv_col=dinv_col,
                             batch_col=batch_col, dinv_bcast=dinv_bcast))

    counts = np.bincount(batch, minlength=G).astype(np.float32)
    cinv = (1.0 / np.maximum(counts, 1.0)).astype(np.float32)

    meta = dict(N=N, P=P, S=S, T=T, SP=SP, NP=NP, HALF=HALF, G=G,
                C_tot=C_tot, idx_cols=idx_cols, tiles=tiles)
    return meta, per_core, cinv


def _pack_cpack(meta, pc, params, cinv):
    """Pack all small f32 inputs into one [128, CK] tensor; returns (arr, off)."""
    T, SP, C_tot, G = meta["T"], meta["SP"], meta["C_tot"], meta["G"]
    cols = {}
    pos = 0

    def add(name, n):
        nonlocal pos
        cols[name] = pos
        pos += n

    add("ident", 128)
    add("iota128", 128)
    add("iotaG", G)
    add("w0", 128)
    add("w1", 128)
    add("w2", 128)
    add("wout", 1)
    add("g0", 1); add("be0", 1); add("g1", 1); add("be1", 1)
    add("g2", 1); add("be2", 1)
    add("cinv", 1)
    add("boutv", 1)
    add("dinv_col", T)
    add("batch_col", T)
    add("dinv_bcast", SP)
    add("slots", C_tot)
    CK = pos

    a = np.zeros((128, CK), dtype=np.float32)
    a[:, cols["ident"]:cols["ident"] + 128] = np.eye(128, dtype=np.float32)
    a[:, cols["iota128"]:cols["iota128"] + 128] = np.broadcast_to(
        np.arange(128, dtype=np.float32), (128, 128))
    a[:, cols["iotaG"]:cols["iotaG"] + G] = np.broadcast_to(
        np.arange(G, dtype=np.float32), (128, G))
    for i in range(3):
        a[:, cols[f"w{i}"]:cols[f"w{i}"] + 128] = np.asarray(params[f"W{i}"],
                                                             np.float32)
        a[:, cols[f"g{i}"]] = np.asarray(params[f"g{i}"], np.float32)
        a[:, cols[f"be{i}"]] = np.asarray(params[f"be{i}"], np.float32)
    a[:, cols["wout"]] = np.asarray(params["Wout"], np.float32)[:, 0]
    a[:meta["G"], cols["cinv"]] = cinv
    a[:meta["G"], cols["boutv"]] = float(np.asarray(params["bout"],
                                                    np.float32)[0])
    a[:, cols["dinv_col"]:cols["dinv_col"] + T] = pc["dinv_col"]
    a[:, cols["batch_col"]:cols["batch_col"] + T] = pc["batch_col"]
    a[:, cols["dinv_bcast"]:cols["dinv_bcast"] + SP] = pc["dinv_bcast"]
    a[:, cols["slots"]:cols["slots"] + C_tot] = pc["slots"]
    return a, cols, CK


# ---------------------------------------------------------------------------
# device program
# ---------------------------------------------------------------------------

def build_nc(meta, cols, CK, reps=1, no_cc=False, no_gather=False, nq=4):
    N, P, T, SP, NP, HALF, G = (meta[k] for k in
                                ("N", "P", "T", "SP", "NP", "HALF", "G"))
    tiles = meta["tiles"]
    C_tot, idx_cols = meta["C_tot"], meta["idx_cols"]
    Cmax = max(ca + cb for ca, cb, *_ in tiles)
    # SWDGE descriptor ring holds ~1024 descs; cap idxs per dma_gather call
    GCAP = 7  # chunks of 128 idxs per call
    calls_per_tile = [-(-ca // GCAP) + -(-cb // GCAP)
                      for ca, cb, *_ in tiles]

    nc = bacc.Bacc("TRN2", num_devices=P, num_swdge_queues=nq)
    rg = [list(range(P))]

    cpack_d = nc.declare_dram_parameter("cpack", [128, CK], F32, isOutput=False)
    xs_d = nc.declare_dram_parameter("xs", [SP, H], F32, isOutput=False)
    idxs_d = nc.declare_dram_parameter("idxs", [128, idx_cols], I16,
                                       isOutput=False)
    out_d = nc.declare_dram_parameter("out", [G, 1], F32, isOutput=True)

    cc_in = nc.dram_tensor("cc_in", [SP, 128], BF)
    mfull = nc.dram_tensor("mfull", [NP, 128], BF, addr_space="Shared")
    ar_in = nc.dram_tensor("ar_in", [128, 2], F32)
    ar_out = nc.dram_tensor("ar_out", [128, 2], F32, addr_space="Shared")
    ar2_in = nc.dram_tensor("ar2_in", [128, G], F32)
    ar2_out = nc.dram_tensor("ar2_out", [128, G], F32, addr_space="Shared")

    # XOR-hypercube allreduce (recursive doubling) over remote_dma_broadcast:
    # step s partners with core ^ (1<<s); relative dest slot 1<<s satisfies
    # the D2D slot rule (bit2 of slot == bit2 of delta-tpb). All exchanges on
    # SWDGE queue 3 (gathers use 0-2). The per-layer CC AllGather is a global
    # barrier, so single-buffered step bufs can't be overwritten early.
    RD = [[None] * 8 for _ in range(3)]
    for s in range(3):
        RD[s][1 << s] = (0, 1 << s)

    import contextlib
    es = contextlib.ExitStack()

    def sb(name, shape, dt):
        return es.enter_context(nc.sbuf_tensor(name, shape, dt))

    def ps(name, shape):
        return es.enter_context(nc.psum_tensor(name, shape, F32))

    def sem(name):
        return es.enter_context(nc.semaphore(name))

    with es:
        cp = sb("cp", [128, CK], F32)
        xsb = sb("xsb", [128, T * H], F32)
        hT = sb("hT", [128, SP], F32)
        uT = sb("uT", [128, SP], F32)
        mps = sb("mps", [128, T * 128], BF)
        idxs_sb = sb("idxs_sb", [128, idx_cols], I16)
        gath = sb("gath", [128, 3 * Cmax * 128], BF)
        vhot = sb("vhot", [128, 2 * Cmax * 128], BF)
        scr2 = sb("scr2", [128, 2 * 128], F32)
        st1 = sb("st1", [128, T], F32)
        NG8 = -(-T // 8)
        st2 = sb("st2", [128, NG8], F32)
        stp = sb("stp", [128, 2], F32)
        sta = sb("sta", [128, 2], F32)
        hx = sb("hx", [128, 6], F32)        # stats hypercube recv bufs
        ax = sb("ax", [128, 4], F32)        # stats partial sums a0, a1
        px = sb("px", [128, 3 * G], F32)    # pooled hypercube recv bufs
        pax = sb("pax", [128, 2 * G], F32)  # pooled partial sums
        bnp = sb("bnp", [128, 8], F32)
        h3n = sb("h3n", [128, 2 * 128], F32)
        bsel = sb("bsel", [128, 2 * G], F32)
        poos = sb("poos", [128, G], F32)
        pooa = sb("pooa", [128, G], F32)
        outs = sb("outs", [64, 1], F32)

        psA = ps("psA", [128, 2, 512])
        psB = ps("psB", [128, 2, 512])
        psP = ps("psP", [128, 512])
        psO = ps("psO", [128, 512])

        s_in = sem("s_in")
        s_tp = sem("s_tp")
        s_xc = sem("s_xc")
        s_pet = sem("s_pet")
        s_hi = sem("s_hi")
        s_mp = sem("s_mp")
        s_st = sem("s_st")
        s_cc = sem("s_cc")
        s_g = [sem("s_g0"), sem("s_g1"), sem("s_g2")]
        s_vh = sem("s_vh")
        s_pe = sem("s_pe")
        s_uc = sem("s_uc")
        s_uc2 = sem("s_uc2")
        s_sr = sem("s_sr")
        rs = [sem("rs0"), sem("rs1"), sem("rs2")]  # stats arrivals per step
        rp = [sem("rp0"), sem("rp1"), sem("rp2")]  # pooled arrivals per step
        s_pq = sem("s_pq")  # queue-3 desc-prep completion
        s_lq = sem("s_lq")  # queue-3 local send completion (unused waits)
        s_a0 = sem("s_a0")
        s_a1 = sem("s_a1")
        s_pr = sem("s_pr")  # pooled hypercube result ready
        s_bp1 = sem("s_bp1")
        s_bp2 = sem("s_bp2")
        s_bp3 = sem("s_bp3")
        s_bn = sem("s_bn")
        s_tp2 = sem("s_tp2")
        s_h3 = sem("s_h3")
        s_pool = sem("s_pool")
        s_pc = sem("s_pc")
        s_ro = sem("s_ro")
        s_sig = sem("s_sig")

        def col(name, n=1):
            return cp[:, cols[name]:cols[name] + n]

        ident_ap = col("ident", 128)
        iota128_ap = col("iota128", 128)
        iotaG_ap = col("iotaG", G)
        w_ap = [col("w0", 128), col("w1", 128), col("w2", 128)]
        wout_ap = cp[:, cols["wout"]:cols["wout"] + 1]
        cinv_ap = cp[0:G, cols["cinv"]:cols["cinv"] + 1]
        boutv_ap = cp[0:G, cols["boutv"]:cols["boutv"] + 1]

        gath3 = gath[:, :].rearrange("p (s c e) -> p s c e", s=3, e=128)
        vhot3 = vhot[:, :].rearrange("p (s c e) -> p s c e", s=2, e=128)
        mps3 = mps[:, :].rearrange("p (t e) -> p t e", e=128)

        # per-slot gather sem values: slot of global tile gidx = gidx % 3;
        # g_slot_val[gidx] = 16 * (total calls issued on that slot's sem
        # through gidx) — safe wait target under unordered DMA completion.
        RL = 3 * reps  # total layer-slots
        g_slot_val = []
        slot_tally = [0, 0, 0]
        for gidx in range(RL * T):
            slot_tally[gidx % 3] += calls_per_tile[gidx % T]
            g_slot_val.append(16 * slot_tally[gidx % 3])

        def gcalls(C):
            """Split C chunks into dma_gather call spans of <= GCAP chunks."""
            return [(c0, min(c0 + GCAP, C)) for c0 in range(0, C, GCAP)]

        CCI = 16 if no_cc else 1  # s_cc increment per collective

        with nc.Block() as block:

            @block.sync
            def _(sync):
                sync.dma_start(out=cp[:, :], in_=cpack_d[:, :]).then_inc(s_in, 16)
                sync.dma_start(
                    out=xsb[:, :],
                    in_=xs_d.ap().rearrange("(p t) f -> p (t f)", p=128),
                ).then_inc(s_in, 16)
                sync.dma_start(out=idxs_sb[:, :], in_=idxs_d[:, :]).then_inc(
                    s_in, 16)
                for rep in range(reps):
                    for l in range(3):
                        LG = rep * 3 + l
                        # M' store -> cc_in
                        sync.wait_ge(s_mp, (LG + 1) * T)
                        sync.dma_start(
                            out=cc_in.ap().rearrange("(p t) f -> p (t f)",
                                                     p=128),
                            in_=mps[:, :],
                        ).then_inc(s_st, 16)
                        # stats store
                        sync.wait_ge(s_sr, LG + 1)
                        sync.dma_start(out=ar_in[:, :],
                                       in_=stp[:, :]).then_inc(s_st, 16)
                        # stats load back after AR
                        sync.wait_ge(s_cc, (rep * 7 + 2 * l + 2) * CCI)
                        sync.dma_start(out=sta[:, :],
                                       in_=ar_out[:, :]).then_inc(s_pr, 16)
                    # pooled store
                    sync.wait_ge(s_pc, rep + 1)
                    sync.dma_start(out=ar2_in[:, :], in_=poos[:, :]).then_inc(
                        s_st, 16)
                    # pooled load after AR
                    sync.wait_ge(s_cc, (rep * 7 + 7) * CCI)
                    sync.dma_start(out=pooa[:, :],
                                   in_=ar2_out[:, :]).then_inc(s_pr, 16)
                    # final out
                    sync.wait_ge(s_sig, rep + 1)
                    sync.dma_start(out=out_d[:, :], in_=outs[:, :]).then_inc(
                        s_st, 16)

            def _cc(gpsimd, kind, op, ins, outs):
                """Collective, or (no_cc timing mode) a local stand-in DMA."""
                if no_cc:
                    return gpsimd.dma_start(
                        out=outs[0].tensor[0:ins[0].shape[0], :],
                        in_=ins[0])
                return gpsimd.collective_compute(
                    kind, op, replica_groups=rg, ins=ins, outs=outs)

            @block.gpsimd
            def _(gpsimd):
                gpsimd.load_library(_mlp_lib)
                gpsimd.wait_ge(s_in, 48)
                def hleg(gpsimd, hz, li, wsem, wval, src, dst, rsem):
                    """One hypercube leg: wait src ready, prep bcast, fire."""
                    gpsimd.wait_ge(wsem, wval)
                    gpsimd.remote_dma_broadcast(
                        out_ap=dst, in_ap=src, remote_sem=rsem[li],
                        local_sem=s_lq, rdests=RD[li], queue_num=3,
                    ).then_inc(s_pq, 1)
                    gpsimd.wait_ge(s_pq, 3 * hz + li + 1)
                    gpsimd.trigger_dma(1, queue_num=3)

                for rep in range(reps):
                    for l in range(3):
                        LG = rep * 3 + l
                        # AllGather M'
                        gpsimd.wait_ge(s_st, rep * 128 + 32 * l + 16)
                        _cc(gpsimd, "AllGather", ALU.bypass,
                            [cc_in[:, :]], [mfull[:, :]]).then_inc(s_cc, CCI)
                        gpsimd.wait_ge(s_cc, (rep * 7 + 2 * l + 1) * CCI)
                        for t in range(T):
                            gidx = LG * T + t
                            # one wait per 3-tile group: s_pe >= gidx at
                            # gidx%3==0 covers slots gidx, gidx+1, gidx+2
                            # (each needs only s_pe >= gidx-2..gidx)
                            if gidx >= 3 and gidx % 3 == 0:
                                gpsimd.wait_ge(s_pe, gidx)
                            CAt, CBt, icolA, icolB, _, _ = tiles[t]
                            gslot = gidx % 3
                            for (C, ic, cb, base) in (
                                (CAt, icolA, 0, 0),
                                (CBt, icolB, CAt, HALF),
                            ):
                                for (c0, c1) in gcalls(C):
                                    if no_gather:
                                        continue
                                    gpsimd.dma_gather(
                                        gath3[:, gslot, cb + c0:cb + c1, :],
                                        mfull[base:base + HALF, :],
                                        idxs_sb[:, ic + c0 * 8:ic + c1 * 8],
                                        (c1 - c0) * 128,
                                        (c1 - c0) * 128,
                                        128,
                                        queue_num=gslot,
                                    ).then_inc(s_g[gslot], 16)
                        # AllReduce stats
                        gpsimd.wait_ge(s_st, rep * 128 + 32 * l + 32)
                        _cc(gpsimd, "AllReduce", ALU.add,
                            [ar_in[:, :]], [ar_out[:, :]]).then_inc(s_cc, CCI)
                    # pooled AllReduce
                    gpsimd.wait_ge(s_st, rep * 128 + 112)
                    _cc(gpsimd, "AllReduce", ALU.add,
                        [ar2_in[:, :]], [ar2_out[:, :]]).then_inc(s_cc, CCI)

            @block.tensor
            def _(tensor):
                tensor.wait_ge(s_in, 48)
                for rep in range(reps):
                    PB = rep * 5 * T  # psB global sequence base
                    # x-prep: transpose x tiles into psB, DVE copies to hT
                    for t in range(T):
                        if t < 2:
                            if rep > 0:
                                tensor.wait_ge(s_h3, rep * T)
                        else:
                            tensor.wait_ge(s_xc, rep * T + t - 1)
                        tensor.matmul(
                            psB[:, (PB + t) % 2, 0:128],
                            xsb[:, t * H:(t + 1) * H],
                            ident_ap,
                            is_transpose=True,
                        ).then_inc(s_tp, 1)
                    for l in range(3):
                        LG = rep * 3 + l
                        # transform: t_tile = hT_tile.T @ W (node-major psB)
                        for t in range(T):
                            midx = LG * T + t
                            pb = PB + T + l * T + t
                            if l == 0 and t == 0:
                                tensor.wait_ge(s_xc, (rep + 1) * T)
                            if l > 0 and t == 0:
                                tensor.wait_ge(s_bn, LG)
                            if midx >= 2 and midx % 2 == 0:
                                # covers this tile (needs midx-1) AND the
                                # next odd tile (needs midx)
                                tensor.wait_ge(s_mp, midx)
                            tensor.matmul(
                                psB[:, pb % 2, 0:128],
                                hT[:, t * H:(t + 1) * H],
                                w_ap[l],
                            ).then_inc(s_pet, 1)
                        # scatter
                        for t in range(T):
                            gidx = LG * T + t
                            CAt, CBt, *_ = tiles[t]
                            Ct = CAt + CBt
                            if not no_gather:
                                tensor.wait_ge(s_g[gidx % 3],
                                               g_slot_val[gidx])
                            # s_vh >= gidx+1 transitively implies s_uc >= gidx-1
                            # (vhot(gidx) is emitted after ucopy(gidx-2) on DVE)
                            tensor.wait_ge(s_vh, gidx + 1)
                            gslot = gidx % 3
                            vslot = gidx % 2
                            for c in range(Ct):
                                mm = tensor.matmul(
                                    psA[:, gidx % 2, 0:128],
                                    gath3[:, gslot, c, 0:128],
                                    vhot3[:, vslot, c, :],
                                    start=(c == 0), stop=(c == Ct - 1),
                                )
                            mm.then_inc(s_pe, 1)
                    # pooling
                    for t in range(T):
                        pb = PB + 4 * T + t
                        if t == 0:
                            tensor.wait_ge(s_bn, rep * 3 + 3)
                        if t >= 2:
                            tensor.wait_ge(s_h3, rep * T + t - 1)
                        tensor.matmul(
                            psB[:, pb % 2, 0:128],
                            hT[:, t * H:(t + 1) * H],
                            ident_ap,
                            is_transpose=True,
                        ).then_inc(s_tp2, 1)
                        if t >= 1:
                            tensor.wait_ge(s_h3, rep * T + t)
                            tensor.matmul(
                                psP[:, 0:G],
                                h3n[:, 128 * ((t - 1) % 2):
                                    128 * ((t - 1) % 2) + 128],
                                bsel[:, G * ((t - 1) % 2):
                                     G * ((t - 1) % 2) + G],
                                start=(t == 1), stop=False,
                            )
                    tensor.wait_ge(s_h3, (rep + 1) * T)
                    tensor.matmul(
                        psP[:, 0:G],
                        h3n[:, 128 * ((T - 1) % 2):128 * ((T - 1) % 2) + 128],
                        bsel[:, G * ((T - 1) % 2):G * ((T - 1) % 2) + G],
                        start=(T == 1), stop=True,
                    ).then_inc(s_pool, 1)
                    # readout matmul
                    tensor.wait_ge(s_pr, rep * 64 + 64)
                    tensor.matmul(psO[0:G, 0:1], pooa[:, 0:G],
                                  wout_ap).then_inc(s_ro, 1)

            @block.scalar
            def _(scalar):
                for rep in range(reps):
                    PB = rep * 5 * T
                    for l in range(3):
                        LG = rep * 3 + l
                        for t in range(T):
                            midx = LG * T + t
                            pb = PB + T + l * T + t
                            scalar.wait_ge(s_pet, midx + 1)
                            scalar.activation(
                                mps3[:, t, 0:128],
                                psB[:, pb % 2, 0:128],
                                AF.Copy,
                                scale=col("dinv_col", T)[:, t:t + 1],
                            ).then_inc(s_mp, 1)
                        # scatter phase: sumsq per GROUP of 8 tiles (u by DVE);
                        # scratch goes to mps, which is dead during scatter
                        # (M' already stored to DRAM before the AllGather).
                        for gi, t0 in enumerate(range(0, T, 8)):
                            t1 = min(t0 + 8, T)
                            scalar.wait_ge(s_uc, LG * T + t1)
                            scalar.activation(
                                mps3[:, t0:t1, 0:128],
                                uT[:, t0 * H:t1 * H].rearrange(
                                    "p (t f) -> p t f", f=H),
                                AF.Square,
                                accum_out=st2[:, gi:gi + 1],
                            ).then_inc(s_uc2, 1)
                        # BN tail: sqrt, A = sd*g, mA = mu*A
                        scalar.wait_ge(s_bp1, LG + 1)
                        scalar.activation(bnp[:, 5:6], bnp[:, 4:5], AF.Sqrt)
                        scalar.drain()
                        scalar.activation(bnp[:, 6:7], bnp[:, 5:6], AF.Copy,
                                          scale=col(f"g{l}"))
                        scalar.drain()
                        scalar.activation(bnp[:, 7:8], bnp[:, 6:7], AF.Copy,
                                          scale=bnp[:, 0:1]).then_inc(s_bp2, 1)
                        # BN+relu big op
                        scalar.wait_ge(s_bp3, LG + 1)
                        scalar.activation(
                            hT[:, :], uT[:, :], AF.Relu,
                            bias=bnp[:, 3:4], scale=bnp[:, 6:7],
                        ).then_inc(s_bn, 1)
                    # sigmoid readout
                    scalar.wait_ge(s_ro, rep + 1)
                    scalar.activation(outs[:, :], psO[0:G, 0:1], AF.Sigmoid,
                                      bias=boutv_ap, scale=cinv_ap).then_inc(
                                          s_sig, 1)

            @block.vector
            def _(vector):
                vector.wait_ge(s_in, 48)
                for rep in range(reps):
                    PB = rep * 5 * T
                    # x-prep copies psB -> hT
                    for t in range(T):
                        vector.wait_ge(s_tp, rep * T + t + 1)
                        vector.tensor_copy(
                            hT[:, t * H:(t + 1) * H],
                            psB[:, (PB + t) % 2, 0:128]).then_inc(s_xc, 1)
                    for l in range(3):
                        LG = rep * 3 + l
                        # scatter phase: vhot prologue
                        base_pe = LG * T
                        for pv in range(min(2, T)):
                            gidx = base_pe + pv
                            if gidx >= 2:
                                vector.wait_ge(s_pe, gidx - 1)
                            CAt, CBt, _, _, ccolA, _ = tiles[pv]
                            Ct = CAt + CBt
                            vector.tensor_tensor(
                                vhot3[:, gidx % 2, 0:Ct, :],
                                _ap3(iota128_ap, 1, (0, Ct)),
                                _ap3(col("slots", C_tot)[:, ccolA:ccolA + Ct],
                                     2, (0, 128)),
                                op=ALU.is_equal,
                            ).then_inc(s_vh, 1)
                        for t in range(T):
                            gidx = base_pe + t
                            vector.wait_ge(s_pe, gidx + 1)
                            # u = psum * dinv, accumulate sum
                            vector.scalar_tensor_tensor(
                                uT[:, t * H:(t + 1) * H],
                                psA[:, gidx % 2, 0:128],
                                1.0,
                                col("dinv_bcast", SP)[:, t * H:(t + 1) * H],
                                op0=ALU.mult, op1=ALU.mult,
                                accum_out=st1[:, t:t + 1],
                            ).then_inc(s_uc, 1)
                            if t + 2 < T:
                                nt = t + 2
                                CAt, CBt, _, _, ccolA, _ = tiles[nt]
                                Ct = CAt + CBt
                                vector.tensor_tensor(
                                    vhot3[:, (base_pe + nt) % 2, 0:Ct, :],
                                    _ap3(iota128_ap, 1, (0, Ct)),
                                    _ap3(col("slots", C_tot)[:,
                                         ccolA:ccolA + Ct], 2, (0, 128)),
                                    op=ALU.is_equal,
                                ).then_inc(s_vh, 1)
                        # stats reduce
                        vector.drain()
                        vector.wait_ge(s_uc2, (LG + 1) * NG8)
                        vector.reduce_sum(stp[:, 0:1], st1[:, :],
                                          axis=mybir.AxisListType.X)
                        vector.reduce_sum(stp[:, 1:2], st2[:, 0:NG8],
                                          axis=mybir.AxisListType.X).then_inc(
                                              s_sr, 1)
                        # BN params from AR result
                        vector.wait_ge(s_pr, rep * 64 + 16 * (l + 1))
                        vector.tensor_scalar_mul(bnp[:, 0:1], sta[:, 0:1],
                                                 1.0 / N)
                        vector.tensor_scalar_mul(bnp[:, 1:2], sta[:, 1:2],
                                                 1.0 / N)
                        vector.drain()
                        vector.tensor_mul(bnp[:, 2:3], bnp[:, 0:1],
                                          bnp[:, 0:1])
                        vector.drain()
                        vector.tensor_sub(bnp[:, 2:3], bnp[:, 1:2],
                                          bnp[:, 2:3])
                        vector.drain()
                        vector.tensor_scalar_add(bnp[:, 2:3], bnp[:, 2:3],
                                                 EPS)
                        vector.drain()
                        vector.reciprocal(bnp[:, 4:5], bnp[:, 2:3]).then_inc(
                            s_bp1, 1)
                        # B = be - mu*A (scalar computed mA in bnp[:,7:8])
                        vector.wait_ge(s_bp2, LG + 1)
                        vector.tensor_sub(bnp[:, 3:4], col(f"be{l}"),
                                          bnp[:, 7:8]).then_inc(s_bp3, 1)
                    # pooling: copy transposes + bsel
                    for t in range(T):
                        vector.wait_ge(s_tp2, rep * T + t + 1)
                        pb = PB + 4 * T + t
                        vector.tensor_copy(
                            h3n[:, 128 * (t % 2):128 * (t % 2) + 128],
                            psB[:, pb % 2, 0:128])
                        vector.tensor_scalar(
                            bsel[:, G * (t % 2):G * (t % 2) + G],
                            iotaG_ap,
                            col("batch_col", T)[:, t:t + 1],
                            None,
                            op0=ALU.is_equal,
                        ).then_inc(s_h3, 1)
                    vector.wait_ge(s_pool, rep + 1)
                    vector.tensor_copy(poos[:, :],
                                       psP[:, 0:G]).then_inc(s_pc, 1)
                    vector.drain()

        nc.compile()
    return nc


# ---------------------------------------------------------------------------
# entry point
# ---------------------------------------------------------------------------

def kernel(**inputs):
    x = np.asarray(inputs["x"], np.float32)
    edge_index = np.asarray(inputs["edge_index"])
    batch = np.asarray(inputs["batch"])
    G = 64
    P = P_CORES

    meta, per_core, cinv = _prep(x, edge_index, batch, P, G)
    in_maps = []
    cols = CK = None
    for c in range(P):
        cpack, cols, CK = _pack_cpack(meta, per_core[c], inputs, cinv)
        in_maps.append({
            "cpack": cpack,
            "xs": per_core[c]["xs"],
            "idxs": per_core[c]["idxs"],
        })

    nc = build_nc(meta, cols, CK)

    if os.environ.get("GCN_SIM"):
        from concourse import bass_interp
        sim = bass_interp.MultiCoreSim(nc, P)
        for c in range(P):
            for k, v in in_maps[c].items():
                sim.cores[c].tensor(k)[:] = v
        sim.simulate()
        return np.asarray(sim.cores[0].mem_tensor("out"), np.float32)

    from concourse.bass_utils import run_bass_kernel_spmd
    trace = bool(os.environ.get("GCN_TRACE"))
    res = run_bass_kernel_spmd(nc, in_maps, core_ids=list(range(P)),
                               trace=trace)
    if trace and res.exec_time_ns is not None:
        print(f"HW exec time: {res.exec_time_ns} ns")
    return np.asarray(res.results[0]["out"], np.float32)



# revision 31
# speedup vs baseline: 654.5392x; 1.1064x over previous
"""Trainium2 Bass kernel for a 3-layer GCN binary graph classifier.

Self-contained: takes the FULL inputs of reference.setup_inputs(), shards
across 8 NeuronCores internally, returns the FULL [64,1] output.

Algorithm (validated against the jax reference; bf16 message table keeps
the final rel err ~1e-4, far inside the 2e-2 gate):
  - destination-node sharding: core c owns nodes [c*6250, (c+1)*6250)
  - per layer: M' = dinv * (h @ W) computed on the owner core (fp32 PSUM),
    rounded to bf16 and AllGathered into a shared DRAM table [NP, 128]bf16
  - each core dma_gathers its in-edges' rows (256B/edge) across 3 SWDGE
    queues (slot gidx%3 -> queue gidx%3; a DMA-completion semaphore is
    locked to one queue) and scatter-adds via one-hot matmuls (one-hot
    built on DVE by iota==slot compare, bf16), feature-major PSUM
    [128f x 128dst]
  - u = dinv[dst] * psum; BN batch stats via tiny AllReduce; BN+ReLU fused
    on the scalar engine (the conv bias b cancels exactly inside BN)
  - mean-pool via one-hot matmul per graph + AllReduce; sigmoid readout.
"""

import dataclasses
import os

import numpy as np
import ml_dtypes

import concourse.bass as bass
import concourse.bacc as bacc
import concourse.mybir as mybir
from concourse.library_config import mlp as _mlp_lib

BF16 = ml_dtypes.bfloat16
F32 = mybir.dt.float32
BF = mybir.dt.bfloat16
I16 = mybir.dt.int16
AF = mybir.ActivationFunctionType
ALU = mybir.AluOpType

H = 128
EPS = 1e-5
P_CORES = 8


def _ap3(ap, ins_at, pair):
    """Insert [step,count] pair into a 2D AP at position ins_at (1 or 2)."""
    aps = list(ap.ap)
    aps.insert(ins_at, list(pair))
    return dataclasses.replace(ap, ap=aps)


# ---------------------------------------------------------------------------
# host prep
# ---------------------------------------------------------------------------

def _prep(x, edge_index, batch, P, G):
    N, D = x.shape
    assert D == H and N % P == 0
    S = N // P
    T = (S + 127) // 128
    SP = T * 128
    NP = P * SP
    HALF = NP // 2
    assert HALF - 1 < 32768

    src = np.asarray(edge_index[0], dtype=np.int64)
    dst = np.asarray(edge_index[1], dtype=np.int64)
    batch = np.asarray(batch, dtype=np.int64)

    deg = np.bincount(dst, minlength=N).astype(np.float32) + 1.0
    dinv = (1.0 / np.sqrt(deg)).astype(np.float32)

    nodes = np.arange(N, dtype=np.int64)
    owner = nodes // S
    ln = nodes - owner * S
    rowmap = owner * SP + (ln % 128) * T + (ln // 128)

    src_all = np.concatenate([src, nodes])
    dst_all = np.concatenate([dst, nodes])
    srow = rowmap[src_all]
    half = (srow >= HALF).astype(np.int64)

    c_of = dst_all // S
    ld = dst_all - c_of * S
    t_of = ld // 128
    slot_of = ld % 128

    key = ((c_of * T) + t_of) * 2 + half
    order = np.argsort(key, kind="stable")
    key_s = key[order]
    srow_s = srow[order]
    slot_s = slot_of[order]

    ngroups = P * T * 2
    starts = np.searchsorted(key_s, np.arange(ngroups))
    ends = np.searchsorted(key_s, np.arange(ngroups) + 1)
    cnt = (ends - starts).reshape(P, T, 2)

    CA = np.ceil(cnt[:, :, 0].max(axis=0) / 128).astype(np.int64)
    CB = np.ceil(cnt[:, :, 1].max(axis=0) / 128).astype(np.int64)
    C_tot = int((CA + CB).sum())
    idx_cols = C_tot * 8

    tiles = []
    icol = 0
    ccol = 0
    for t in range(T):
        rec = []
        for hh, C in ((0, int(CA[t])), (1, int(CB[t]))):
            rec.append((C, icol, ccol))
            icol += C * 8
            ccol += C
        tiles.append((rec[0][0], rec[1][0], rec[0][1], rec[1][1],
                      rec[0][2], rec[1][2]))

    per_core = []
    for c in range(P):
        idxs = np.zeros((128, idx_cols), dtype=np.int16)
        slots = np.full((128, C_tot), 1000.0, dtype=np.float32)
        for t in range(T):
            CAt, CBt, icolA, icolB, ccolA, ccolB = tiles[t]
            for hh, C, ic, cc in ((0, CAt, icolA, ccolA), (1, CBt, icolB, ccolB)):
                if C == 0:
                    continue
                g = starts[(c * T + t) * 2 + hh]
                e = ends[(c * T + t) * 2 + hh]
                n = e - g
                arr_i = np.zeros(C * 128, dtype=np.int64)
                arr_s = np.full(C * 128, 1000.0, dtype=np.float32)
                arr_i[:n] = srow_s[g:e] - (HALF if hh else 0)
                arr_s[:n] = slot_s[g:e]
                w = arr_i.reshape(C * 8, 16).T.astype(np.int16)
                idxs[:, ic:ic + C * 8] = np.tile(w, (8, 1))
                slots[:, cc:cc + C] = arr_s.reshape(C, 128).T

        xst = np.zeros((SP, D), dtype=np.float32)
        lnn = np.arange(S)
        xst[(lnn % 128) * T + lnn // 128] = np.asarray(x[c * S:(c + 1) * S],
                                                       np.float32)
        dinv_col = np.zeros((128, T), dtype=np.float32)
        batch_col = np.full((128, T), float(G), dtype=np.float32)
        dsh = dinv[c * S:(c + 1) * S]
        bsh = batch[c * S:(c + 1) * S].astype(np.float32)
        dinv_col[lnn % 128, lnn // 128] = dsh
        batch_col[lnn % 128, lnn // 128] = bsh
        dinv_b = np.zeros((SP,), dtype=np.float32)
        dinv_b[:S] = dsh
        dinv_bcast = np.broadcast_to(dinv_b, (128, SP)).copy()

        per_core.append(dict(idxs=idxs, slots=slots, xs=xst, dinv_col=dinv_col,
                             batch_col=batch_col, dinv_bcast=dinv_bcast))

    counts = np.bincount(batch, minlength=G).astype(np.float32)
    cinv = (1.0 / np.maximum(counts, 1.0)).astype(np.float32)

    meta = dict(N=N, P=P, S=S, T=T, SP=SP, NP=NP, HALF=HALF, G=G,
                C_tot=C_tot, idx_cols=idx_cols, tiles=tiles)
    return meta, per_core, cinv


def _pack_cpack(meta, pc, params, cinv):
    """Pack all small f32 inputs into one [128, CK] tensor; returns (arr, off)."""
    T, SP, C_tot, G = meta["T"], meta["SP"], meta["C_tot"], meta["G"]
    cols = {}
    pos = 0

    def add(name, n):
        nonlocal pos
        cols[name] = pos
        pos += n

    add("ident", 128)
    add("iota128", 128)
    add("iotaG", G)
    add("w0", 128)
    add("w1", 128)
    add("w2", 128)
    add("wout", 1)
    add("g0", 1); add("be0", 1); add("g1", 1); add("be1", 1)
    add("g2", 1); add("be2", 1)
    add("cinv", 1)
    add("boutv", 1)
    add("dinv_col", T)
    add("batch_col", T)
    add("dinv_bcast", SP)
    add("slots", C_tot)
    CK = pos

    a = np.zeros((128, CK), dtype=np.float32)
    a[:, cols["ident"]:cols["ident"] + 128] = np.eye(128, dtype=np.float32)
    a[:, cols["iota128"]:cols["iota128"] + 128] = np.broadcast_to(
        np.arange(128, dtype=np.float32), (128, 128))
    a[:, cols["iotaG"]:cols["iotaG"] + G] = np.broadcast_to(
        np.arange(G, dtype=np.float32), (128, G))
    for i in range(3):
        a[:, cols[f"w{i}"]:cols[f"w{i}"] + 128] = np.asarray(params[f"W{i}"],
                                                             np.float32)
        a[:, cols[f"g{i}"]] = np.asarray(params[f"g{i}"], np.float32)
        a[:, cols[f"be{i}"]] = np.asarray(params[f"be{i}"], np.float32)
    a[:, cols["wout"]] = np.asarray(params["Wout"], np.float32)[:, 0]
    a[:meta["G"], cols["cinv"]] = cinv
    a[:meta["G"], cols["boutv"]] = float(np.asarray(params["bout"],
                                                    np.float32)[0])
    a[:, cols["dinv_col"]:cols["dinv_col"] + T] = pc["dinv_col"]
    a[:, cols["batch_col"]:cols["batch_col"] + T] = pc["batch_col"]
    a[:, cols["dinv_bcast"]:cols["dinv_bcast"] + SP] = pc["dinv_bcast"]
    a[:, cols["slots"]:cols["slots"] + C_tot] = pc["slots"]
    return a, cols, CK


# ---------------------------------------------------------------------------
# device program
# ---------------------------------------------------------------------------

def build_nc(meta, cols, CK, reps=1, no_cc=False, no_gather=False, nq=4,
             sp=False):
    N, P, T, SP, NP, HALF, G = (meta[k] for k in
                                ("N", "P", "T", "SP", "NP", "HALF", "G"))
    tiles = meta["tiles"]
    C_tot, idx_cols = meta["C_tot"], meta["idx_cols"]
    Cmax = max(ca + cb for ca, cb, *_ in tiles)
    # SWDGE descriptor ring holds ~1024 descs; cap idxs per dma_gather call
    GCAP = 7  # chunks of 128 idxs per call
    calls_per_tile = [-(-ca // GCAP) + -(-cb // GCAP)
                      for ca, cb, *_ in tiles]

    nc = bacc.Bacc("TRN2", num_devices=P, num_swdge_queues=nq)
    rg = [list(range(P))]

    cpack_d = nc.declare_dram_parameter("cpack", [128, CK], F32, isOutput=False)
    xs_d = nc.declare_dram_parameter("xs", [SP, H], F32, isOutput=False)
    idxs_d = nc.declare_dram_parameter("idxs", [128, idx_cols], I16,
                                       isOutput=False)
    out_d = nc.declare_dram_parameter("out", [G, 1], F32, isOutput=True)

    cc_in = nc.dram_tensor("cc_in", [SP, 128], BF)
    mfull = nc.dram_tensor("mfull", [NP, 128], BF, addr_space="Shared")
    ar_in = nc.dram_tensor("ar_in", [128, 2], F32)
    ar_out = nc.dram_tensor("ar_out", [128, 2], F32, addr_space="Shared")
    ar2_in = nc.dram_tensor("ar2_in", [128, G], F32)
    ar2_out = nc.dram_tensor("ar2_out", [128, G], F32, addr_space="Shared")

    # XOR-hypercube allreduce (recursive doubling) over remote_dma_broadcast:
    # step s partners with core ^ (1<<s); relative dest slot 1<<s satisfies
    # the D2D slot rule (bit2 of slot == bit2 of delta-tpb). All exchanges on
    # SWDGE queue 3 (gathers use 0-2). The per-layer CC AllGather is a global
    # barrier, so single-buffered step bufs can't be overwritten early.
    RD = [[None] * 8 for _ in range(3)]
    for s in range(3):
        RD[s][1 << s] = (0, 1 << s)

    import contextlib
    es = contextlib.ExitStack()

    def sb(name, shape, dt):
        return es.enter_context(nc.sbuf_tensor(name, shape, dt))

    def ps(name, shape):
        return es.enter_context(nc.psum_tensor(name, shape, F32))

    def sem(name):
        return es.enter_context(nc.semaphore(name))

    with es:
        cp = sb("cp", [128, CK], F32)
        xsb = sb("xsb", [128, T * H], F32)
        hT = sb("hT", [128, SP], F32)
        uT = sb("uT", [128, SP], F32)
        mps = sb("mps", [128, T * 128], BF)
        idxs_sb = sb("idxs_sb", [128, idx_cols], I16)
        gath = sb("gath", [128, 3 * Cmax * 128], BF)
        vhot = sb("vhot", [128, 2 * Cmax * 128], BF)
        scr2 = sb("scr2", [128, 2 * 128], F32)
        st1 = sb("st1", [128, T], F32)
        NG8 = -(-T // 8)
        st2 = sb("st2", [128, NG8], F32)
        stp = sb("stp", [128, 2], F32)
        sta = sb("sta", [128, 2], F32)
        hx = sb("hx", [128, 6], F32)        # stats hypercube recv bufs
        ax = sb("ax", [128, 4], F32)        # stats partial sums a0, a1
        px = sb("px", [128, 3 * G], F32)    # pooled hypercube recv bufs
        pax = sb("pax", [128, 2 * G], F32)  # pooled partial sums
        bnp = sb("bnp", [128, 8], F32)
        h3n = sb("h3n", [128, 2 * 128], F32)
        bsel = sb("bsel", [128, 2 * G], F32)
        poos = sb("poos", [128, G], F32)
        pooa = sb("pooa", [128, G], F32)
        outs = sb("outs", [64, 1], F32)

        psA = ps("psA", [128, 2, 512])
        psB = ps("psB", [128, 2, 512])
        psP = ps("psP", [128, 512])
        psO = ps("psO", [128, 512])

        s_in = sem("s_in")
        s_tp = sem("s_tp")
        s_xc = sem("s_xc")
        s_pet = sem("s_pet")
        s_hi = sem("s_hi")
        s_mp = sem("s_mp")
        s_st = sem("s_st")
        s_cc = sem("s_cc")
        s_g = [sem("s_g0"), sem("s_g1"), sem("s_g2")]
        s_vh = sem("s_vh")
        s_pe = sem("s_pe")
        s_uc = sem("s_uc")
        s_uc2 = sem("s_uc2")
        s_sr = sem("s_sr")
        rs = [sem("rs0"), sem("rs1"), sem("rs2")]  # stats arrivals per step
        rp = [sem("rp0"), sem("rp1"), sem("rp2")]  # pooled arrivals per step
        s_pq = sem("s_pq")  # queue-3 desc-prep completion
        s_lq = sem("s_lq")  # queue-3 local send completion (unused waits)
        s_a0 = sem("s_a0")
        s_a1 = sem("s_a1")
        s_pr = sem("s_pr")  # pooled hypercube result ready
        s_bp1 = sem("s_bp1")
        s_bp2 = sem("s_bp2")
        s_bp3 = sem("s_bp3")
        s_bn = sem("s_bn")
        s_tp2 = sem("s_tp2")
        s_h3 = sem("s_h3")
        s_pool = sem("s_pool")
        s_pc = sem("s_pc")
        s_ro = sem("s_ro")
        s_sig = sem("s_sig")

        def col(name, n=1):
            return cp[:, cols[name]:cols[name] + n]

        ident_ap = col("ident", 128)
        iota128_ap = col("iota128", 128)
        iotaG_ap = col("iotaG", G)
        w_ap = [col("w0", 128), col("w1", 128), col("w2", 128)]
        wout_ap = cp[:, cols["wout"]:cols["wout"] + 1]
        cinv_ap = cp[0:G, cols["cinv"]:cols["cinv"] + 1]
        boutv_ap = cp[0:G, cols["boutv"]:cols["boutv"] + 1]

        gath3 = gath[:, :].rearrange("p (s c e) -> p s c e", s=3, e=128)
        vhot3 = vhot[:, :].rearrange("p (s c e) -> p s c e", s=2, e=128)
        mps3 = mps[:, :].rearrange("p (t e) -> p t e", e=128)

        # per-slot gather sem values: slot of global tile gidx = gidx % 3;
        # g_slot_val[gidx] = 16 * (total calls issued on that slot's sem
        # through gidx) — safe wait target under unordered DMA completion.
        RL = 3 * reps  # total layer-slots
        g_slot_val = []
        slot_tally = [0, 0, 0]
        for gidx in range(RL * T):
            slot_tally[gidx % 3] += calls_per_tile[gidx % T]
            g_slot_val.append(16 * slot_tally[gidx % 3])

        def gcalls(C):
            """Split C chunks into dma_gather call spans of <= GCAP chunks."""
            return [(c0, min(c0 + GCAP, C)) for c0 in range(0, C, GCAP)]

        CCI = 16 if no_cc else 1  # s_cc increment per collective

        with nc.Block() as block:

            @block.sync
            def _(sync):
                sync.dma_start(out=cp[:, :], in_=cpack_d[:, :]).then_inc(s_in, 16)
                sync.dma_start(
                    out=xsb[:, :],
                    in_=xs_d.ap().rearrange("(p t) f -> p (t f)", p=128),
                ).then_inc(s_in, 16)
                sync.dma_start(out=idxs_sb[:, :], in_=idxs_d[:, :]).then_inc(
                    s_in, 16)
                for rep in range(reps):
                    for l in range(3):
                        LG = rep * 3 + l
                        # M' store -> cc_in
                        sync.wait_ge(s_mp, (LG + 1) * T)
                        sync.dma_start(
                            out=cc_in.ap().rearrange("(p t) f -> p (t f)",
                                                     p=128),
                            in_=mps[:, :],
                        ).then_inc(s_st, 16)
                        # stats store
                        sync.wait_ge(s_sr, LG + 1)
                        sync.dma_start(out=ar_in[:, :],
                                       in_=stp[:, :]).then_inc(s_st, 16)
                        # stats load back after AR
                        sync.wait_ge(s_cc, (rep * 7 + 2 * l + 2) * CCI)
                        sync.dma_start(out=sta[:, :],
                                       in_=ar_out[:, :]).then_inc(s_pr, 16)
                    # pooled store
                    sync.wait_ge(s_pc, rep + 1)
                    sync.dma_start(out=ar2_in[:, :], in_=poos[:, :]).then_inc(
                        s_st, 16)
                    # pooled load after AR
                    sync.wait_ge(s_cc, (rep * 7 + 7) * CCI)
                    sync.dma_start(out=pooa[:, :],
                                   in_=ar2_out[:, :]).then_inc(s_pr, 16)
                    # final out
                    sync.wait_ge(s_sig, rep + 1)
                    sync.dma_start(out=out_d[:, :], in_=outs[:, :]).then_inc(
                        s_st, 16)

            def _cc(gpsimd, kind, op, ins, outs):
                """Collective, or (no_cc timing mode) a local stand-in DMA."""
                if no_cc:
                    return gpsimd.dma_start(
                        out=outs[0].tensor[0:ins[0].shape[0], :],
                        in_=ins[0])
                return gpsimd.collective_compute(
                    kind, op, replica_groups=rg, ins=ins, outs=outs)

            @block.gpsimd
            def _(gpsimd):
                gpsimd.load_library(_mlp_lib)
                gpsimd.wait_ge(s_in, 48)
                def hleg(gpsimd, hz, li, wsem, wval, src, dst, rsem):
                    """One hypercube leg: wait src ready, prep bcast, fire."""
                    gpsimd.wait_ge(wsem, wval)
                    gpsimd.remote_dma_broadcast(
                        out_ap=dst, in_ap=src, remote_sem=rsem[li],
                        local_sem=s_lq, rdests=RD[li], queue_num=3,
                    ).then_inc(s_pq, 1)
                    gpsimd.wait_ge(s_pq, 3 * hz + li + 1)
                    gpsimd.trigger_dma(1, queue_num=3)

                for rep in range(reps):
                    for l in range(3):
                        LG = rep * 3 + l
                        # AllGather M'
                        gpsimd.wait_ge(s_st, rep * 128 + 32 * l + 16)
                        _cc(gpsimd, "AllGather", ALU.bypass,
                            [cc_in[:, :]], [mfull[:, :]]).then_inc(s_cc, CCI)
                        gpsimd.wait_ge(s_cc, (rep * 7 + 2 * l + 1) * CCI)
                        for t in range(T):
                            gidx = LG * T + t
                            # one wait per 3-tile group: s_pe >= gidx at
                            # gidx%3==0 covers slots gidx, gidx+1, gidx+2
                            # (each needs only s_pe >= gidx-2..gidx)
                            if gidx >= 3 and gidx % 3 == 0:
                                gpsimd.wait_ge(s_pe, gidx)
                            CAt, CBt, icolA, icolB, _, _ = tiles[t]
                            gslot = gidx % 3
                            for (C, ic, cb, base) in (
                                (CAt, icolA, 0, 0),
                                (CBt, icolB, CAt, HALF),
                            ):
                                for (c0, c1) in gcalls(C):
                                    if no_gather:
                                        continue
                                    gpsimd.dma_gather(
                                        gath3[:, gslot, cb + c0:cb + c1, :],
                                        mfull[base:base + HALF, :],
                                        idxs_sb[:, ic + c0 * 8:ic + c1 * 8],
                                        (c1 - c0) * 128,
                                        (c1 - c0) * 128,
                                        128,
                                        queue_num=gslot,
                                        single_packet=sp,
                                    ).then_inc(s_g[gslot], 16)
                        # AllReduce stats
                        gpsimd.wait_ge(s_st, rep * 128 + 32 * l + 32)
                        _cc(gpsimd, "AllReduce", ALU.add,
                            [ar_in[:, :]], [ar_out[:, :]]).then_inc(s_cc, CCI)
                    # pooled AllReduce
                    gpsimd.wait_ge(s_st, rep * 128 + 112)
                    _cc(gpsimd, "AllReduce", ALU.add,
                        [ar2_in[:, :]], [ar2_out[:, :]]).then_inc(s_cc, CCI)

            @block.tensor
            def _(tensor):
                tensor.wait_ge(s_in, 48)
                for rep in range(reps):
                    PB = rep * 5 * T  # psB global sequence base
                    # x-prep: transpose x tiles into psB, DVE copies to hT
                    for t in range(T):
                        if t < 2:
                            if rep > 0:
                                tensor.wait_ge(s_h3, rep * T)
                        else:
                            tensor.wait_ge(s_xc, rep * T + t - 1)
                        tensor.matmul(
                            psB[:, (PB + t) % 2, 0:128],
                            xsb[:, t * H:(t + 1) * H],
                            ident_ap,
                            is_transpose=True,
                        ).then_inc(s_tp, 1)
                    for l in range(3):
                        LG = rep * 3 + l
                        # transform: t_tile = hT_tile.T @ W (node-major psB)
                        for t in range(T):
                            midx = LG * T + t
                            pb = PB + T + l * T + t
                            if l == 0 and t == 0:
                                tensor.wait_ge(s_xc, (rep + 1) * T)
                            if l > 0 and t == 0:
                                tensor.wait_ge(s_bn, LG)
                            if midx >= 2 and midx % 2 == 0:
                                # covers this tile (needs midx-1) AND the
                                # next odd tile (needs midx)
                                tensor.wait_ge(s_mp, midx)
                            tensor.matmul(
                                psB[:, pb % 2, 0:128],
                                hT[:, t * H:(t + 1) * H],
                                w_ap[l],
                            ).then_inc(s_pet, 1)
                        # scatter
                        for t in range(T):
                            gidx = LG * T + t
                            CAt, CBt, *_ = tiles[t]
                            Ct = CAt + CBt
                            if not no_gather:
                                tensor.wait_ge(s_g[gidx % 3],
                                               g_slot_val[gidx])
                            # s_vh >= gidx+1 transitively implies s_uc >= gidx-1
                            # (vhot(gidx) is emitted after ucopy(gidx-2) on DVE)
                            tensor.wait_ge(s_vh, gidx + 1)
                            gslot = gidx % 3
                            vslot = gidx % 2
                            for c in range(Ct):
                                mm = tensor.matmul(
                                    psA[:, gidx % 2, 0:128],
                                    gath3[:, gslot, c, 0:128],
                                    vhot3[:, vslot, c, :],
                                    start=(c == 0), stop=(c == Ct - 1),
                                )
                            mm.then_inc(s_pe, 1)
                    # pooling
                    for t in range(T):
                        pb = PB + 4 * T + t
                        if t == 0:
                            tensor.wait_ge(s_bn, rep * 3 + 3)
                        if t >= 2:
                            tensor.wait_ge(s_h3, rep * T + t - 1)
                        tensor.matmul(
                            psB[:, pb % 2, 0:128],
                            hT[:, t * H:(t + 1) * H],
                            ident_ap,
                            is_transpose=True,
                        ).then_inc(s_tp2, 1)
                        if t >= 1:
                            tensor.wait_ge(s_h3, rep * T + t)
                            tensor.matmul(
                                psP[:, 0:G],
                                h3n[:, 128 * ((t - 1) % 2):
                                    128 * ((t - 1) % 2) + 128],
                                bsel[:, G * ((t - 1) % 2):
                                     G * ((t - 1) % 2) + G],
                                start=(t == 1), stop=False,
                            )
                    tensor.wait_ge(s_h3, (rep + 1) * T)
                    tensor.matmul(
                        psP[:, 0:G],
                        h3n[:, 128 * ((T - 1) % 2):128 * ((T - 1) % 2) + 128],
                        bsel[:, G * ((T - 1) % 2):G * ((T - 1) % 2) + G],
                        start=(T == 1), stop=True,
                    ).then_inc(s_pool, 1)
                    # readout matmul
                    tensor.wait_ge(s_pr, rep * 64 + 64)
                    tensor.matmul(psO[0:G, 0:1], pooa[:, 0:G],
                                  wout_ap).then_inc(s_ro, 1)

            @block.scalar
            def _(scalar):
                for rep in range(reps):
                    PB = rep * 5 * T
                    for l in range(3):
                        LG = rep * 3 + l
                        for t in range(T):
                            midx = LG * T + t
                            pb = PB + T + l * T + t
                            scalar.wait_ge(s_pet, midx + 1)
                            scalar.activation(
                                mps3[:, t, 0:128],
                                psB[:, pb % 2, 0:128],
                                AF.Copy,
                                scale=col("dinv_col", T)[:, t:t + 1],
                            ).then_inc(s_mp, 1)
                        # scatter phase: sumsq per GROUP of 8 tiles (u by DVE);
                        # scratch goes to mps, which is dead during scatter
                        # (M' already stored to DRAM before the AllGather).
                        for gi, t0 in enumerate(range(0, T, 8)):
                            t1 = min(t0 + 8, T)
                            scalar.wait_ge(s_uc, LG * T + t1)
                            scalar.activation(
                                mps3[:, t0:t1, 0:128],
                                uT[:, t0 * H:t1 * H].rearrange(
                                    "p (t f) -> p t f", f=H),
                                AF.Square,
                                accum_out=st2[:, gi:gi + 1],
                            ).then_inc(s_uc2, 1)
                        # BN tail: sqrt, A = sd*g, mA = mu*A
                        scalar.wait_ge(s_bp1, LG + 1)
                        scalar.activation(bnp[:, 5:6], bnp[:, 4:5], AF.Sqrt)
                        scalar.drain()
                        scalar.activation(bnp[:, 6:7], bnp[:, 5:6], AF.Copy,
                                          scale=col(f"g{l}"))
                        scalar.drain()
                        scalar.activation(bnp[:, 7:8], bnp[:, 6:7], AF.Copy,
                                          scale=bnp[:, 0:1]).then_inc(s_bp2, 1)
                        # BN+relu big op
                        scalar.wait_ge(s_bp3, LG + 1)
                        scalar.activation(
                            hT[:, :], uT[:, :], AF.Relu,
                            bias=bnp[:, 3:4], scale=bnp[:, 6:7],
                        ).then_inc(s_bn, 1)
                    # sigmoid readout
                    scalar.wait_ge(s_ro, rep + 1)
                    scalar.activation(outs[:, :], psO[0:G, 0:1], AF.Sigmoid,
                                      bias=boutv_ap, scale=cinv_ap).then_inc(
                                          s_sig, 1)

            @block.vector
            def _(vector):
                vector.wait_ge(s_in, 48)
                for rep in range(reps):
                    PB = rep * 5 * T
                    # x-prep copies psB -> hT
                    for t in range(T):
                        vector.wait_ge(s_tp, rep * T + t + 1)
                        vector.tensor_copy(
                            hT[:, t * H:(t + 1) * H],
                            psB[:, (PB + t) % 2, 0:128]).then_inc(s_xc, 1)
                    for l in range(3):
                        LG = rep * 3 + l
                        # scatter phase: vhot prologue
                        base_pe = LG * T
                        for pv in range(min(2, T)):
                            gidx = base_pe + pv
                            if gidx >= 2:
                                vector.wait_ge(s_pe, gidx - 1)
                            CAt, CBt, _, _, ccolA, _ = tiles[pv]
                            Ct = CAt + CBt
                            vector.tensor_tensor(
                                vhot3[:, gidx % 2, 0:Ct, :],
                                _ap3(iota128_ap, 1, (0, Ct)),
                                _ap3(col("slots", C_tot)[:, ccolA:ccolA + Ct],
                                     2, (0, 128)),
                                op=ALU.is_equal,
                            ).then_inc(s_vh, 1)
                        for t in range(T):
                            gidx = base_pe + t
                            vector.wait_ge(s_pe, gidx + 1)
                            # u = psum * dinv, accumulate sum
                            vector.scalar_tensor_tensor(
                                uT[:, t * H:(t + 1) * H],
                                psA[:, gidx % 2, 0:128],
                                1.0,
                                col("dinv_bcast", SP)[:, t * H:(t + 1) * H],
                                op0=ALU.mult, op1=ALU.mult,
                                accum_out=st1[:, t:t + 1],
                            ).then_inc(s_uc, 1)
                            if t + 2 < T:
                                nt = t + 2
                                CAt, CBt, _, _, ccolA, _ = tiles[nt]
                                Ct = CAt + CBt
                                vector.tensor_tensor(
                                    vhot3[:, (base_pe + nt) % 2, 0:Ct, :],
                                    _ap3(iota128_ap, 1, (0, Ct)),
                                    _ap3(col("slots", C_tot)[:,
                                         ccolA:ccolA + Ct], 2, (0, 128)),
                                    op=ALU.is_equal,
                                ).then_inc(s_vh, 1)
                        # stats reduce
                        vector.drain()
                        vector.wait_ge(s_uc2, (LG + 1) * NG8)
                        vector.reduce_sum(stp[:, 0:1], st1[:, :],
                                          axis=mybir.AxisListType.X)
                        vector.reduce_sum(stp[:, 1:2], st2[:, 0:NG8],
                                          axis=mybir.AxisListType.X).then_inc(
                                              s_sr, 1)
                        # BN params from AR result
                        vector.wait_ge(s_pr, rep * 64 + 16 * (l + 1))
                        vector.tensor_scalar_mul(bnp[:, 0:1], sta[:, 0:1],
                                                 1.0 / N)
                        vector.tensor_scalar_mul(bnp[:, 1:2], sta[:, 1:2],
                                                 1.0 / N)
                        vector.drain()
                        vector.tensor_mul(bnp[:, 2:3], bnp[:, 0:1],
                                          bnp[:, 0:1])
                        vector.drain()
                        vector.tensor_sub(bnp[:, 2:3], bnp[:, 1:2],
                                          bnp[:, 2:3])
                        vector.drain()
                        vector.tensor_scalar_add(bnp[:, 2:3], bnp[:, 2:3],
                                                 EPS)
                        vector.drain()
                        vector.reciprocal(bnp[:, 4:5], bnp[:, 2:3]).then_inc(
                            s_bp1, 1)
                        # B = be - mu*A (scalar computed mA in bnp[:,7:8])
                        vector.wait_ge(s_bp2, LG + 1)
                        vector.tensor_sub(bnp[:, 3:4], col(f"be{l}"),
                                          bnp[:, 7:8]).then_inc(s_bp3, 1)
                    # pooling: copy transposes + bsel
                    for t in range(T):
                        vector.wait_ge(s_tp2, rep * T + t + 1)
                        pb = PB + 4 * T + t
                        vector.tensor_copy(
                            h3n[:, 128 * (t % 2):128 * (t % 2) + 128],
                            psB[:, pb % 2, 0:128])
                        vector.tensor_scalar(
                            bsel[:, G * (t % 2):G * (t % 2) + G],
                            iotaG_ap,
                            col("batch_col", T)[:, t:t + 1],
                            None,
                            op0=ALU.is_equal,
                        ).then_inc(s_h3, 1)
                    vector.wait_ge(s_pool, rep + 1)
                    vector.tensor_copy(poos[:, :],
                                       psP[:, 0:G]).then_inc(s_pc, 1)
                    vector.drain()

        nc.compile()
    return nc


# ---------------------------------------------------------------------------
# entry point
# ---------------------------------------------------------------------------

def kernel(**inputs):
    x = np.asarray(inputs["x"], np.float32)
    edge_index = np.asarray(inputs["edge_index"])
    batch = np.asarray(inputs["batch"])
    G = 64
    P = P_CORES

    meta, per_core, cinv = _prep(x, edge_index, batch, P, G)
    in_maps = []
    cols = CK = None
    for c in range(P):
        cpack, cols, CK = _pack_cpack(meta, per_core[c], inputs, cinv)
        in_maps.append({
            "cpack": cpack,
            "xs": per_core[c]["xs"],
            "idxs": per_core[c]["idxs"],
        })

    nc = build_nc(meta, cols, CK)

    if os.environ.get("GCN_SIM"):
        from concourse import bass_interp
        sim = bass_interp.MultiCoreSim(nc, P)
        for c in range(P):
            for k, v in in_maps[c].items():
                sim.cores[c].tensor(k)[:] = v
        sim.simulate()
        return np.asarray(sim.cores[0].mem_tensor("out"), np.float32)

    from concourse.bass_utils import run_bass_kernel_spmd
    trace = bool(os.environ.get("GCN_TRACE"))
    res = run_bass_kernel_spmd(nc, in_maps, core_ids=list(range(P)),
                               trace=trace)
    if trace and res.exec_time_ns is not None:
        print(f"HW exec time: {res.exec_time_ns} ns")
    return np.asarray(res.results[0]["out"], np.float32)

